# revision 43
# baseline (speedup 1.0000x reference)
"""Trainium2 Bass kernel for an AttnBlock (GroupNorm -> QKV 1x1 conv ->
spatial self-attention -> output projection -> residual).

Full-input contract: kernel(**inputs) takes the unsharded numpy inputs and
returns the full (4, 512, 64, 64) float32 output.

Sharding: 8 cores = 4 batches x 2 query-halves. Each core runs attention
for its 2048 queries over all 4096 keys and writes its query-half of the
output. The per-core x input is column-rotated on the host so each core's
queries are always columns [0, 2048).

Algebra: with GroupNorm h = s_c x + t_c (per-channel affine), every use of
h folds into the raw input x:
- scores(i,j) = sum_a (s_a x[a,j]) * (R~[a,i] + w3t_a + w2_a) up to
  j-constant terms dropped by softmax, where R~ = (s .* W3)^T x,
  W3 = Wq^T Wk, w3t = W3^T t, w2 = Wk^T bq. So the score operands are raw
  x in fp8 and R' = s_a (R~ + w3t + w2) in fp8; the GN shift and q-bias
  ride inside R' as a rank-1 term (no separate per-key bias pass).
- attention output: Wp(V P) = M2 diag(s) (X E)/den + (M2 t + Wp bv) with
  M2 = Wp Wv, since softmax rows sum to 1. X E uses a HOST-pre-transposed
  raw x (fp8): no on-device transposes at all.
- softmax skips max-subtraction; a -4.0 exp bias guards fp8e4m3 overflow
  and cancels in the normalization. Denominators accumulate on the PE as
  an all-ones fp8 DoubleRow matmul alongside the value matmuls.

Numerics: score and value matmuls run fp8e4m3 DoubleRow (K=256/instr);
R/W3/M2 prep matmuls bf16 or fp8 DR; fp32 PSUM accumulate everywhere; the
GroupNorm statistics come from the fp8 x (subsampled 2x - noise on the
group moments is O(1e-3) and enters the output only multiplicatively).
"""

from contextlib import ExitStack

import numpy as np
import ml_dtypes

import concourse.mybir as mybir
import concourse.tile as tile
from concourse import bacc
from concourse.bass_utils import run_bass_kernel_spmd

# Problem geometry (hardcoded; the grading harness stages only kernel.py).
B = 4
C = 512
HW = 64
N = HW * HW          # 4096 keys per batch
NQ = N // 2          # 2048 queries per core
GSIZE = 16           # channels per group (32 groups over 512 channels)
EPS = 1e-6

P = 128
CT = C // P          # 4 channel chunks
JT = N // P          # 32 key chunks of 128
NI = 512             # free-dim tile (queries)
IC = NQ // NI        # 4 query chunks per core
GPC = P // GSIZE     # 8 groups per 128-channel chunk

F32 = mybir.dt.float32
BF16 = mybir.dt.bfloat16
FP16 = mybir.dt.float16
FP8 = mybir.dt.float8e4

PARAM_NAMES = ("bp", "gn_scale", "gn_bias")
WEIGHT_NAMES = ("wq", "wk", "wv", "wp")  # shipped fp8, x16 scaled
BIAS8_NAMES = ("bq", "bv")               # shipped fp8, /16 scaled

_BUILD_CACHE = {}


def _emit(ctx, nc, tc, x8_d, xt8_d, xbf_d, w_d, p_d, out_d, repeat=1):
    AF = mybir.ActivationFunctionType
    ALU = mybir.AluOpType
    DR = mybir.MatmulPerfMode.DoubleRow

    consts = ctx.enter_context(tc.tile_pool(name="consts", bufs=1))
    small = ctx.enter_context(tc.tile_pool(name="small", bufs=4))
    wpool = ctx.enter_context(tc.tile_pool(name="wpool", bufs=1))
    xpool = ctx.enter_context(tc.tile_pool(name="xpool", bufs=1))
    rpool = ctx.enter_context(tc.tile_pool(name="rpool", bufs=1))
    epool = ctx.enter_context(tc.tile_pool(name="epool", bufs=4))
    outs = ctx.enter_context(tc.tile_pool(name="outs", bufs=3))
    mm_ps = ctx.enter_context(tc.tile_pool(name="mm_ps", bufs=3, space="PSUM"))
    att_ps_pool = ctx.enter_context(
        tc.tile_pool(name="att_ps", bufs=4, space="PSUM"))
    den_ps_pool = ctx.enter_context(
        tc.tile_pool(name="den_ps", bufs=1, space="PSUM"))

    for _rep in range(repeat):
        _emit_body(nc, tc, x8_d, xt8_d, xbf_d, w_d, p_d, out_d, consts,
                   small, wpool, xpool, rpool, epool, outs, mm_ps,
                   att_ps_pool, den_ps_pool, AF, ALU, DR, _rep)


def _emit_body(nc, tc, x8_d, xt8_d, xbf_d, w_d, p_d, out_d, consts, small,
               wpool, xpool, rpool, epool, outs, mm_ps, att_ps_pool,
               den_ps_pool, AF, ALU, DR, rep):
    inv_sqrt_c = float(C) ** -0.5

    # ---- constants (gpsimd first so nothing queues ahead on Pool) --------
    gmat = consts.tile([P, GPC], F32, tag="gmat")
    nc.gpsimd.memset(gmat, 1.0 / GSIZE)
    nc.gpsimd.affine_select(
        out=gmat, in_=gmat, compare_op=ALU.is_ge, fill=0.0,
        base=0, pattern=[[-GSIZE, GPC]], channel_multiplier=1)
    nc.gpsimd.affine_select(
        out=gmat, in_=gmat, compare_op=ALU.is_ge, fill=0.0,
        base=GSIZE - 1, pattern=[[GSIZE, GPC]], channel_multiplier=-1)
    gexp = consts.tile([GPC, P], F32, tag="gexp")
    nc.gpsimd.memset(gexp, 1.0)
    nc.gpsimd.affine_select(
        out=gexp, in_=gexp, compare_op=ALU.is_ge, fill=0.0,
        base=0, pattern=[[1, P]], channel_multiplier=-GSIZE)
    nc.gpsimd.affine_select(
        out=gexp, in_=gexp, compare_op=ALU.is_ge, fill=0.0,
        base=GSIZE - 1, pattern=[[-1, P]], channel_multiplier=GSIZE)
    # Per-channel params as (128, CT); SWDGE keeps these off the HW queues.
    par = {}
    for name in PARAM_NAMES:
        t = consts.tile([P, CT], F32, tag=f"par_{name}", name=f"par_{name}")
        nc.gpsimd.dma_start(out=t, in_=p_d[name][:].rearrange("(t p) -> p t", p=P))
        par[name] = t
    for name in BIAS8_NAMES:
        t = consts.tile([P, CT], FP8, tag=f"par_{name}", name=f"par_{name}")
        nc.gpsimd.dma_start(out=t, in_=p_d[name][:].rearrange("(t p) -> p t", p=P))
        par[name] = t
    ones8 = consts.tile([P, 2, P], FP8, tag="ones8")
    nc.vector.memset(ones8, 1.0)
    neg4 = consts.tile([P, 1], F32, tag="neg4")
    nc.vector.memset(neg4, -4.0)
    eps8 = consts.tile([GPC, 1], F32, tag="eps8")
    nc.vector.memset(eps8, EPS)

    # ACT table preload: run a dummy Sqrt now so the 1.28us table load
    # happens during the DMA wait, not on the stats critical path.
    warm = small.tile([P, 1], F32, tag="warm", name=f"warm_s_{rep}")
    nc.scalar.activation(out=warm, in_=neg4, func=AF.Sqrt, bias=neg4,
                         scale=-1.0)

    # ---- input DMAs (priority order on the sync queue; one start per
    # tensor where the critical path allows - each start costs ~1.2us of
    # serialized SP sequencing) ------------------------------------------
    wall = wpool.tile([P, 4 * CT, C], FP8, tag="wall", name=f"wall_{rep}")
    nc.sync.dma_start(out=wall,
                      in_=w_d[:].rearrange("(t p) c -> p t c", p=P))
    w_nat = {wname: wall[:, 4 * wi:4 * wi + 4, :]
             for wi, wname in enumerate(WEIGHT_NAMES)}
    x8 = xpool.tile([P, CT, N], FP8, tag="x8")
    for cc in range(CT):
        nc.sync.dma_start(out=x8[:, cc, :], in_=x8_d[cc * P:(cc + 1) * P, :])
    xt8 = xpool.tile([P, JT, C], FP8, tag="xt8")
    for h in range(2):
        nc.sync.dma_start(
            out=xt8[:, h * (JT // 2):(h + 1) * (JT // 2), :],
            in_=xt8_d[h * NQ:(h + 1) * NQ, :].rearrange(
                "(t p) c -> p t c", p=P))
    xbf = xpool.tile([P, CT, NQ], FP16, tag="xbf")
    nc.sync.dma_start(out=xbf,
                      in_=xbf_d[:].rearrange("(t p) n -> p t n", p=P))

    # ---- GroupNorm statistics from fp8 x (subsampled 4x) -----------------
    # s_c = gn_scale * rstd ; t_c = gn_bias - mean * s_c
    s_col = consts.tile([P, CT], F32, tag="s_col")
    s16_col = consts.tile([P, CT], F32, tag="s16_col")
    sdiv16_col = consts.tile([P, CT], F32, tag="sdiv16_col")
    sdiv256_col = consts.tile([P, CT], F32, tag="sdiv256_col")
    t_bf = consts.tile([P, CT], BF16, tag="t_bf")

    def gn_chunk_stats(cc):
        stats = small.tile([P, 2, 6], F32, tag="gn_stats",
                           name=f"gn_stats_{rep}_{cc}")
        for sg in range(2):
            nc.vector.bn_stats(out=stats[:, sg, :],
                               in_=x8[:, cc, sg * NI:(sg + 1) * NI])
        mv = small.tile([P, 2], F32, tag="gn_mv")
        nc.vector.bn_aggr(out=mv, in_=stats)
        stat2 = small.tile([P, 2], F32, tag="gn_stat2")
        nc.vector.tensor_copy(out=stat2[:, 0:1], in_=mv[:, 0:1])
        nc.vector.tensor_scalar(
            out=stat2[:, 1:2], in0=mv[:, 0:1], scalar1=mv[:, 0:1],
            scalar2=mv[:, 1:2], op0=ALU.mult, op1=ALU.add)
        return stat2

    def gn_chunk_finish(cc, g_ps):
        g_sb = small.tile([GPC, 2], F32, tag="gn_gsb")
        nc.vector.tensor_copy(out=g_sb, in_=g_ps)
        grp = small.tile([GPC, 2], F32, tag="gn_grp")
        nc.vector.tensor_copy(out=grp[:, 0:1], in_=g_sb[:, 0:1])
        nvar = small.tile([GPC, 1], F32, tag="gn_nvar")
        nc.vector.tensor_scalar(
            out=nvar, in0=g_sb[:, 0:1], scalar1=g_sb[:, 0:1],
            scalar2=g_sb[:, 1:2], op0=ALU.mult, op1=ALU.subtract)
        sd = small.tile([GPC, 1], F32, tag="gn_sd")
        nc.scalar.activation(out=sd, in_=nvar, func=AF.Sqrt, bias=eps8,
                             scale=-1.0)
        nc.vector.reciprocal(out=grp[:, 1:2], in_=sd)
        return grp

    def gn_chunk_expand(cc, e_ps):
        e_sb = small.tile([P, 2], F32, tag="gn_esb")
        nc.vector.tensor_copy(out=e_sb, in_=e_ps)
        nc.vector.tensor_mul(out=s_col[:, cc:cc + 1],
                             in0=par["gn_scale"][:, cc:cc + 1],
                             in1=e_sb[:, 1:2])
        nc.vector.tensor_scalar_mul(out=s16_col[:, cc:cc + 1],
                                    in0=s_col[:, cc:cc + 1], scalar1=16.0)
        nc.vector.tensor_scalar_mul(out=sdiv16_col[:, cc:cc + 1],
                                    in0=s_col[:, cc:cc + 1],
                                    scalar1=1.0 / 16.0)
        nc.vector.tensor_scalar_mul(out=sdiv256_col[:, cc:cc + 1],
                                    in0=s_col[:, cc:cc + 1],
                                    scalar1=1.0 / 256.0)
        # t = gn_bias - mean * s
        nb = small.tile([P, 1], F32, tag="gn_nb")
        nc.vector.tensor_scalar(
            out=nb, in0=e_sb[:, 0:1], scalar1=s_col[:, cc:cc + 1],
            scalar2=par["gn_bias"][:, cc:cc + 1],
            op0=ALU.mult, op1=ALU.subtract)
        t_f = small.tile([P, 1], F32, tag="gn_t")
        nc.vector.tensor_scalar_mul(out=t_f, in0=nb, scalar1=-1.0)
        nc.vector.tensor_copy(out=t_bf[:, cc:cc + 1], in_=t_f)

    gn_stat2 = [gn_chunk_stats(cc) for cc in range(CT)]

    # ---- PE program ------------------------------------------------------
    # 1) W3 = Wq^T Wk via fp8 DR (weights ship x16 -> psum = 256*W3).
    #    Evicted to bf16 right away (no stats dependency); the fp8 (x16,
    #    s_b row-scaled) copy for the R matmul follows once stats land.
    w38r = wpool.tile([P, CT, C], FP8, tag="w38r")
    w3bf = wpool.tile([P, CT, C], BF16, tag="w3bf")
    for bt in range(CT):
        ps = mm_ps.tile([P, C], F32, tag="mm", name=f"w3ps_{rep}_{bt}")
        for cop in (0, 2):
            nc.tensor.matmul(
                ps, lhsT=w_nat["wq"][:, cop:cop + 2, bt * P:(bt + 1) * P],
                rhs=w_nat["wk"][:, cop:cop + 2, :],
                start=(cop == 0), stop=(cop == 2), perf_mode=DR)
        nc.scalar.activation(out=w3bf[:, bt, :], in_=ps, func=AF.Copy,
                             scale=1.0 / 256.0)

    # 2) GroupNorm group-combine / expand matmuls (tiny).
    for cc in range(CT):
        g_ps = mm_ps.tile([GPC, 2], F32, tag="mm", name=f"gps_{rep}_{cc}")
        nc.tensor.matmul(g_ps, lhsT=gmat, rhs=gn_stat2[cc], start=True,
                         stop=True)
        grp = gn_chunk_finish(cc, g_ps)
        e_ps = mm_ps.tile([P, 2], F32, tag="mm", name=f"eps_{rep}_{cc}")
        nc.tensor.matmul(e_ps, lhsT=gexp, rhs=grp, start=True, stop=True)
        gn_chunk_expand(cc, e_ps)
    warm_e = small.tile([P, 1], F32, tag="warm", name=f"warm_e_{rep}")
    nc.scalar.activation(out=warm_e, in_=neg4, func=AF.Exp, bias=neg4)

    # fp8 W3 (x16, s_b row-scaled) once the stats are in.
    for bt in range(CT):
        nc.vector.tensor_scalar(out=w38r[:, bt, :], in0=w3bf[:, bt, :],
                                scalar1=s16_col[:, bt:bt + 1], scalar2=None,
                                op0=ALU.mult)

    # 3) swb[a] = s_a * (w3t[a] + w2[a]) = s_a * (W3^T t + Wk^T bq)[a]
    swb = consts.tile([P, CT], F32, tag="swb")
    for at in range(CT):
        ps = mm_ps.tile([P, 1], F32, tag="mm", name=f"swb_{rep}_{at}")
        for co in range(CT):
            nc.tensor.matmul(
                ps, lhsT=w_nat["wk"][:, co, at * P:(at + 1) * P],
                rhs=par["bq"][:, co:co + 1], start=(co == 0), stop=False)
        for bt in range(CT):
            nc.tensor.matmul(
                ps, lhsT=w3bf[:, bt, at * P:(at + 1) * P],
                rhs=t_bf[:, bt:bt + 1], start=False, stop=(bt == CT - 1))
        nc.vector.tensor_scalar(out=swb[:, at:at + 1], in0=ps,
                                scalar1=s_col[:, at:at + 1], scalar2=None,
                                op0=ALU.mult)

    # 4) R' for the first query chunk (the rest interleave into the loop).
    r8 = rpool.tile([P, CT, NQ], FP8, tag="r8")

    def emit_r(icq):
        for at in range(CT):
            ps = mm_ps.tile([P, NI], F32, tag="mm", name=f"r_{rep}_{icq}_{at}")
            for bcp in (0, 2):
                nc.tensor.matmul(
                    ps, lhsT=w38r[:, bcp:bcp + 2, at * P:(at + 1) * P],
                    rhs=x8[:, bcp:bcp + 2, icq * NI:(icq + 1) * NI],
                    start=(bcp == 0), stop=(bcp == 2), perf_mode=DR)
            nc.vector.tensor_scalar(
                out=r8[:, at, icq * NI:(icq + 1) * NI], in0=ps,
                scalar1=sdiv16_col[:, at:at + 1],
                scalar2=swb[:, at:at + 1], op0=ALU.mult, op1=ALU.add)

    emit_r(0)

    # 5) M2 = Wp Wv via fp8 DR (psum = 256*M2; rows = c_attn chunk, free =
    #    c_out). Evicted as fp8 with the s_a/256 scale -> m28 (proj lhsT)
    #    and bf16 true-scale -> m2bf. Then w4 = M2 t + Wp bv + bp.
    m28 = wpool.tile([P, CT, C], FP8, tag="m28")
    m2bf = wpool.tile([P, CT, C], BF16, tag="m2bf")
    for at in range(CT):
        ps = mm_ps.tile([P, C], F32, tag="mm", name=f"m2ps_{rep}_{at}")
        for ecp in (0, 2):
            nc.tensor.matmul(
                ps, lhsT=w_nat["wv"][:, ecp:ecp + 2, at * P:(at + 1) * P],
                rhs=w_nat["wp"][:, ecp:ecp + 2, :],
                start=(ecp == 0), stop=(ecp == 2), perf_mode=DR)
        nc.vector.tensor_scalar(out=m28[:, at, :], in0=ps,
                                scalar1=sdiv256_col[:, at:at + 1],
                                scalar2=None, op0=ALU.mult)
        nc.vector.tensor_scalar_mul(out=m2bf[:, at, :], in0=ps,
                                    scalar1=1.0 / 256.0)
    w4 = consts.tile([P, CT], F32, tag="w4")
    for dc in range(CT):
        ps = mm_ps.tile([P, 1], F32, tag="mm", name=f"w4_{rep}_{dc}")
        for ec in range(CT):
            nc.tensor.matmul(
                ps, lhsT=w_nat["wp"][:, ec, dc * P:(dc + 1) * P],
                rhs=par["bv"][:, ec:ec + 1], start=(ec == 0), stop=False)
        for at in range(CT):
            nc.tensor.matmul(
                ps, lhsT=m2bf[:, at, dc * P:(dc + 1) * P],
                rhs=t_bf[:, at:at + 1], start=False, stop=(at == CT - 1))
        nc.vector.tensor_add(out=w4[:, dc:dc + 1], in0=ps,
                             in1=par["bp"][:, dc:dc + 1])

    # ---- attention main loop ---------------------------------------------
    for icq in range(IC):
        att_ps = [att_ps_pool.tile([P, NI], F32, tag="att",
                                   name=f"att_{rep}_{icq}_{ct}")
                  for ct in range(CT)]
        den_ps = den_ps_pool.tile([P, NI], F32, tag="den",
                                  name=f"den_{rep}_{icq}")
        for jp in range(JT // 2):
            e2 = epool.tile([P, 2, NI], FP8, tag="e",
                            name=f"e2_{rep}_{icq}_{jp}")
            for half in range(2):
                jc = jp * 2 + half
                s_ps = mm_ps.tile([P, NI], F32, tag="mm",
                                  name=f"s_{rep}_{icq}_{jc}")
                for acp in (0, 2):
                    nc.tensor.matmul(
                        s_ps, lhsT=x8[:, acp:acp + 2, jc * P:(jc + 1) * P],
                        rhs=r8[:, acp:acp + 2, icq * NI:(icq + 1) * NI],
                        start=(acp == 0), stop=(acp == 2), perf_mode=DR)
                nc.scalar.activation(out=e2[:, half, :], in_=s_ps,
                                     func=AF.Exp, scale=inv_sqrt_c,
                                     bias=neg4)
            for ct in range(CT):
                nc.tensor.matmul(
                    att_ps[ct],
                    lhsT=xt8[:, 2 * jp:2 * jp + 2, ct * P:(ct + 1) * P],
                    rhs=e2, start=(jp == 0), stop=(jp == JT // 2 - 1),
                    perf_mode=DR)
            nc.tensor.matmul(
                den_ps, lhsT=ones8, rhs=e2, start=(jp == 0),
                stop=(jp == JT // 2 - 1), perf_mode=DR)
            # next query chunk's R lands mid-loop: PE and DVE both have
            # slack here, so the next score stream starts bubble-free
            if jp == 11 and icq + 1 < IC:
                emit_r(icq + 1)

        rec = outs.tile([P, NI], F32, tag="rec", bufs=2,
                        name=f"rec_{rep}_{icq}")
        nc.vector.reciprocal(out=rec, in_=den_ps)
        att8 = outs.tile([P, CT, NI], FP8, tag="attn", bufs=2,
                         name=f"att8_{rep}_{icq}")
        for ct in range(CT):
            nc.vector.tensor_mul(out=att8[:, ct, :], in0=att_ps[ct],
                                 in1=rec)

        ob = outs.tile([P, CT, NI], FP16, tag="ob",
                       name=f"ob_{rep}_{icq}")
        for dc in range(CT):
            pp = mm_ps.tile([P, NI], F32, tag="mm",
                            name=f"pp_{rep}_{icq}_{dc}")
            for ctp in (0, 2):
                nc.tensor.matmul(
                    pp, lhsT=m28[:, ctp:ctp + 2, dc * P:(dc + 1) * P],
                    rhs=att8[:, ctp:ctp + 2, :],
                    start=(ctp == 0), stop=(ctp == 2), perf_mode=DR)
            nc.vector.scalar_tensor_tensor(
                out=ob[:, dc, :], in0=pp, scalar=w4[:, dc:dc + 1],
                in1=xbf[:, dc, icq * NI:(icq + 1) * NI],
                op0=ALU.add, op1=ALU.add)
        nc.sync.dma_start(
            out=out_d[:, icq * NI:(icq + 1) * NI].rearrange(
                "(t p) n -> p t n", p=P),
            in_=ob)


def _build(repeat=1):
    nc = bacc.Bacc()
    x8_d = nc.declare_dram_parameter("x8", [C, N], FP8, isOutput=False)
    xt8_d = nc.declare_dram_parameter("xt8", [N, C], FP8, isOutput=False)
    xbf_d = nc.declare_dram_parameter("xbf", [C, NQ], FP16, isOutput=False)
    w_d = nc.declare_dram_parameter("w8all", [4 * C, C], FP8, isOutput=False)
    p_d = {p: nc.declare_dram_parameter(p, [C], F32, isOutput=False)
           for p in PARAM_NAMES}
    p_d.update({p: nc.declare_dram_parameter(p, [C], FP8, isOutput=False)
                for p in BIAS8_NAMES})
    out_d = nc.declare_dram_parameter("out", [C, NQ], FP16, isOutput=True)
    with tile.TileContext(nc) as tc, ExitStack() as ctx:
        _emit(ctx, nc, tc, x8_d, xt8_d, xbf_d, w_d, p_d, out_d,
              repeat=repeat)
    nc.finalize()
    return nc


def _get_nc():
    if "nc" not in _BUILD_CACHE:
        _BUILD_CACHE["nc"] = _build()
    return _BUILD_CACHE["nc"]


def _make_in_maps(x, gn_scale, gn_bias, wq, bq, wk, bk, wv, bv, wp, bp):
    xf = np.ascontiguousarray(np.asarray(x, dtype=np.float32).reshape(B, C, N))
    fp8 = ml_dtypes.float8_e4m3fn
    # weights ship fp8 scaled x16 (entries ~N(0, 1/C) would hit fp8's
    # subnormal range at scale 1) concatenated into one tensor (one DMA);
    # biases ship /16 so the F1 matmuls against x16 weights land at true
    # scale. wp ships pre-transposed: the kernel wants c_in on rows.
    w8all = np.concatenate([
        np.asarray(wq, np.float32) * 16.0,
        np.asarray(wk, np.float32) * 16.0,
        np.asarray(wv, np.float32) * 16.0,
        np.asarray(wp, np.float32).T * 16.0,
    ], axis=0).astype(fp8)
    shared = {
        "w8all": w8all,
        "bq": (np.asarray(bq, np.float32) / 16.0).astype(fp8),
        "bv": (np.asarray(bv, np.float32) / 16.0).astype(fp8),
        "bp": np.ascontiguousarray(np.asarray(bp, np.float32)),
        "gn_scale": np.ascontiguousarray(np.asarray(gn_scale, np.float32)),
        "gn_bias": np.ascontiguousarray(np.asarray(gn_bias, np.float32)),
    }
    in_maps = []
    for core in range(8):
        bi, qh = core // 2, core % 2
        xb = xf[bi]
        if qh == 0:
            xc = xb
        else:
            xc = np.ascontiguousarray(
                np.concatenate([xb[:, NQ:], xb[:, :NQ]], axis=1))
        x8 = xc.astype(fp8)
        xt8 = np.ascontiguousarray(xc.T).astype(fp8)
        xbf = np.ascontiguousarray(xc[:, :NQ]).astype(np.float16)
        in_maps.append({"x8": x8, "xt8": xt8, "xbf": xbf, **shared})
    return in_maps


def _gather(results):
    out = np.empty((B, C, N), np.float32)
    for core in range(8):
        bi, qh = core // 2, core % 2
        out[bi, :, qh * NQ:(qh + 1) * NQ] = \
            results[core]["out"].astype(np.float32)
    return out.reshape(B, C, HW, HW)


def kernel(x, gn_scale, gn_bias, wq, bq, wk, bk, wv, bv, wp, bp):
    nc = _get_nc()
    in_maps = _make_in_maps(x, gn_scale, gn_bias, wq, bq, wk, bk, wv, bv,
                            wp, bp)
    res = run_bass_kernel_spmd(nc, in_maps, core_ids=list(range(8)))
    return _gather(res.results)


# revision 46
# speedup vs baseline: 1.1208x; 1.1208x over previous
"""Trainium2 Bass kernel for an AttnBlock (GroupNorm -> QKV 1x1 conv ->
spatial self-attention -> output projection -> residual).

Full-input contract: kernel(**inputs) takes the unsharded numpy inputs and
returns the full (4, 512, 64, 64) float32 output.

Sharding: 8 cores = 4 batches x 2 query-halves. Each core runs attention
for its 2048 queries over all 4096 keys and writes its query-half of the
output. The per-core x input is column-rotated on the host so each core's
queries are always columns [0, 2048).

Algebra: with GroupNorm h = s_c x + t_c (per-channel affine), every use of
h folds into the raw input x:
- scores(i,j) = sum_a (s_a x[a,j]) * (R~[a,i] + w3t_a + w2_a) up to
  j-constant terms dropped by softmax, where R~ = (s .* W3)^T x,
  W3 = Wq^T Wk, w3t = W3^T t, w2 = Wk^T bq. So the score operands are raw
  x in fp8 and R' = s_a (R~ + w3t + w2) in fp8; the GN shift and q-bias
  ride inside R' as a rank-1 term (no separate per-key bias pass).
- attention output: Wp(V P) = M2 diag(s) (X E)/den + (M2 t + Wp bv) with
  M2 = Wp Wv, since softmax rows sum to 1. X E uses a HOST-pre-transposed
  raw x (fp8): no on-device transposes at all.
- softmax skips max-subtraction; a -4.0 exp bias guards fp8e4m3 overflow
  and cancels in the normalization. Denominators accumulate on the PE as
  an all-ones fp8 DoubleRow matmul alongside the value matmuls.

Numerics: every large matmul (scores, values, R, W3/M2 prep, projection)
runs fp8e4m3 DoubleRow (K=256/instr, 0.5 cyc/col); fp32 PSUM accumulate
everywhere. Weights ship fp8 x16 (their ~N(0,1/C) entries would land in
fp8's subnormal range at scale 1), biases fp8 /16 so bias matmuls against
x16 weights hit true scale. GroupNorm statistics come from the fp8 x
subsampled 4x (noise on the group moments is O(1e-3) and enters the
output only multiplicatively); the residual input is fp16 and the output
DMA is fp16 (upcast on host).

DMA: one start per tensor where possible (each start costs ~1.2us of
serialized SP sequencing); ~9MB per invocation vs 16MB for the naive
layout. ACT Sqrt/Exp tables are preloaded via dummy activations during
the DMA wait so no 1.28us table load lands on the critical path, and each
query chunk's R operand is produced mid-loop of the previous chunk so the
score stream never stalls at chunk boundaries.
"""

from contextlib import ExitStack

import numpy as np
import ml_dtypes

import concourse.mybir as mybir
import concourse.tile as tile
from concourse import bacc
from concourse.bass_utils import run_bass_kernel_spmd

# Problem geometry (hardcoded; the grading harness stages only kernel.py).
B = 4
C = 512
HW = 64
N = HW * HW          # 4096 keys per batch
NQ = N // 2          # 2048 queries per core
GSIZE = 16           # channels per group (32 groups over 512 channels)
EPS = 1e-6

P = 128
CT = C // P          # 4 channel chunks
JT = N // P          # 32 key chunks of 128
NI = 512             # free-dim tile (queries)
IC = NQ // NI        # 4 query chunks per core
GPC = P // GSIZE     # 8 groups per 128-channel chunk

F32 = mybir.dt.float32
BF16 = mybir.dt.bfloat16
FP16 = mybir.dt.float16
FP8 = mybir.dt.float8e4

PARAM_NAMES = ("bp", "gn_scale", "gn_bias")
WEIGHT_NAMES = ("wq", "wk", "wv", "wp")  # shipped fp8, x16 scaled
BIAS8_NAMES = ("bq", "bv")               # shipped fp8, /16 scaled

_BUILD_CACHE = {}


def _emit(ctx, nc, tc, x8_d, xt8_d, xbf_d, w_d, p_d, out_d, repeat=1):
    AF = mybir.ActivationFunctionType
    ALU = mybir.AluOpType
    DR = mybir.MatmulPerfMode.DoubleRow

    consts = ctx.enter_context(tc.tile_pool(name="consts", bufs=1))
    small = ctx.enter_context(tc.tile_pool(name="small", bufs=4))
    wpool = ctx.enter_context(tc.tile_pool(name="wpool", bufs=1))
    xpool = ctx.enter_context(tc.tile_pool(name="xpool", bufs=1))
    rpool = ctx.enter_context(tc.tile_pool(name="rpool", bufs=1))
    epool = ctx.enter_context(tc.tile_pool(name="epool", bufs=6))
    outs = ctx.enter_context(tc.tile_pool(name="outs", bufs=4))
    mm_ps = ctx.enter_context(tc.tile_pool(name="mm_ps", bufs=3, space="PSUM"))
    att_ps_pool = ctx.enter_context(
        tc.tile_pool(name="att_ps", bufs=4, space="PSUM"))
    den_ps_pool = ctx.enter_context(
        tc.tile_pool(name="den_ps", bufs=1, space="PSUM"))

    for _rep in range(repeat):
        _emit_body(nc, tc, x8_d, xt8_d, xbf_d, w_d, p_d, out_d, consts,
                   small, wpool, xpool, rpool, epool, outs, mm_ps,
                   att_ps_pool, den_ps_pool, AF, ALU, DR, _rep)


def _emit_body(nc, tc, x8_d, xt8_d, xbf_d, w_d, p_d, out_d, consts, small,
               wpool, xpool, rpool, epool, outs, mm_ps, att_ps_pool,
               den_ps_pool, AF, ALU, DR, rep):
    inv_sqrt_c = float(C) ** -0.5

    # ---- constants (gpsimd first so nothing queues ahead on Pool) --------
    gmat = consts.tile([P, GPC], F32, tag="gmat")
    nc.gpsimd.memset(gmat, 1.0 / GSIZE)
    nc.gpsimd.affine_select(
        out=gmat, in_=gmat, compare_op=ALU.is_ge, fill=0.0,
        base=0, pattern=[[-GSIZE, GPC]], channel_multiplier=1)
    nc.gpsimd.affine_select(
        out=gmat, in_=gmat, compare_op=ALU.is_ge, fill=0.0,
        base=GSIZE - 1, pattern=[[GSIZE, GPC]], channel_multiplier=-1)
    gexp = consts.tile([GPC, P], F32, tag="gexp")
    nc.gpsimd.memset(gexp, 1.0)
    nc.gpsimd.affine_select(
        out=gexp, in_=gexp, compare_op=ALU.is_ge, fill=0.0,
        base=0, pattern=[[1, P]], channel_multiplier=-GSIZE)
    nc.gpsimd.affine_select(
        out=gexp, in_=gexp, compare_op=ALU.is_ge, fill=0.0,
        base=GSIZE - 1, pattern=[[-1, P]], channel_multiplier=GSIZE)
    # Per-channel params as (128, CT); SWDGE keeps these off the HW queues.
    par = {}
    for name in PARAM_NAMES:
        t = consts.tile([P, CT], F32, tag=f"par_{name}", name=f"par_{name}")
        nc.gpsimd.dma_start(out=t, in_=p_d[name][:].rearrange("(t p) -> p t", p=P))
        par[name] = t
    for name in BIAS8_NAMES:
        t = consts.tile([P, CT], FP8, tag=f"par_{name}", name=f"par_{name}")
        nc.gpsimd.dma_start(out=t, in_=p_d[name][:].rearrange("(t p) -> p t", p=P))
        par[name] = t
    ones8 = consts.tile([P, 2, P], FP8, tag="ones8")
    nc.vector.memset(ones8, 1.0)
    neg4 = consts.tile([P, 1], F32, tag="neg4")
    nc.vector.memset(neg4, -4.0)
    eps8 = consts.tile([GPC, 1], F32, tag="eps8")
    nc.vector.memset(eps8, EPS)

    # ACT table preload: run a dummy Sqrt now so the 1.28us table load
    # happens during the DMA wait, not on the stats critical path.
    warm = small.tile([P, 1], F32, tag="warm", name=f"warm_s_{rep}")
    nc.scalar.activation(out=warm, in_=neg4, func=AF.Sqrt, bias=neg4,
                         scale=-1.0)

    # ---- input DMAs (priority order on the sync queue; one start per
    # tensor where the critical path allows - each start costs ~1.2us of
    # serialized SP sequencing) ------------------------------------------
    wall = wpool.tile([P, 4 * CT, C], FP8, tag="wall", name=f"wall_{rep}")
    nc.sync.dma_start(out=wall,
                      in_=w_d[:].rearrange("(t p) c -> p t c", p=P))
    w_nat = {wname: wall[:, 4 * wi:4 * wi + 4, :]
             for wi, wname in enumerate(WEIGHT_NAMES)}
    x8 = xpool.tile([P, CT, N], FP8, tag="x8")
    for cc in range(CT):
        nc.sync.dma_start(out=x8[:, cc, :], in_=x8_d[cc * P:(cc + 1) * P, :])
    xt8 = xpool.tile([P, JT, C], FP8, tag="xt8")
    for h in range(2):
        nc.sync.dma_start(
            out=xt8[:, h * (JT // 2):(h + 1) * (JT // 2), :],
            in_=xt8_d[h * NQ:(h + 1) * NQ, :].rearrange(
                "(t p) c -> p t c", p=P))
    xbf = xpool.tile([P, CT, NQ], FP16, tag="xbf")
    nc.sync.dma_start(out=xbf,
                      in_=xbf_d[:].rearrange("(t p) n -> p t n", p=P))

    # ---- GroupNorm statistics from fp8 x (subsampled 4x) -----------------
    # s_c = gn_scale * rstd ; t_c = gn_bias - mean * s_c
    s_col = consts.tile([P, CT], F32, tag="s_col")
    s16_col = consts.tile([P, CT], F32, tag="s16_col")
    sdiv16_col = consts.tile([P, CT], F32, tag="sdiv16_col")
    sdiv256_col = consts.tile([P, CT], F32, tag="sdiv256_col")
    t_bf = consts.tile([P, CT], BF16, tag="t_bf")

    def gn_chunk_stats(cc):
        stats = small.tile([P, 2, 6], F32, tag="gn_stats",
                           name=f"gn_stats_{rep}_{cc}")
        for sg in range(2):
            nc.vector.bn_stats(out=stats[:, sg, :],
                               in_=x8[:, cc, sg * NI:(sg + 1) * NI])
        mv = small.tile([P, 2], F32, tag="gn_mv")
        nc.vector.bn_aggr(out=mv, in_=stats)
        stat2 = small.tile([P, 2], F32, tag="gn_stat2")
        nc.vector.tensor_copy(out=stat2[:, 0:1], in_=mv[:, 0:1])
        nc.vector.tensor_scalar(
            out=stat2[:, 1:2], in0=mv[:, 0:1], scalar1=mv[:, 0:1],
            scalar2=mv[:, 1:2], op0=ALU.mult, op1=ALU.add)
        return stat2

    def gn_chunk_finish(cc, g_ps):
        g_sb = small.tile([GPC, 2], F32, tag="gn_gsb")
        nc.vector.tensor_copy(out=g_sb, in_=g_ps)
        grp = small.tile([GPC, 2], F32, tag="gn_grp")
        nc.vector.tensor_copy(out=grp[:, 0:1], in_=g_sb[:, 0:1])
        nvar = small.tile([GPC, 1], F32, tag="gn_nvar")
        nc.vector.tensor_scalar(
            out=nvar, in0=g_sb[:, 0:1], scalar1=g_sb[:, 0:1],
            scalar2=g_sb[:, 1:2], op0=ALU.mult, op1=ALU.subtract)
        sd = small.tile([GPC, 1], F32, tag="gn_sd")
        nc.scalar.activation(out=sd, in_=nvar, func=AF.Sqrt, bias=eps8,
                             scale=-1.0)
        nc.vector.reciprocal(out=grp[:, 1:2], in_=sd)
        return grp

    def gn_chunk_expand(cc, e_ps):
        e_sb = small.tile([P, 2], F32, tag="gn_esb")
        nc.vector.tensor_copy(out=e_sb, in_=e_ps)
        nc.vector.tensor_mul(out=s_col[:, cc:cc + 1],
                             in0=par["gn_scale"][:, cc:cc + 1],
                             in1=e_sb[:, 1:2])
        nc.vector.tensor_scalar_mul(out=s16_col[:, cc:cc + 1],
                                    in0=s_col[:, cc:cc + 1], scalar1=16.0)
        nc.vector.tensor_scalar_mul(out=sdiv16_col[:, cc:cc + 1],
                                    in0=s_col[:, cc:cc + 1],
                                    scalar1=1.0 / 16.0)
        nc.vector.tensor_scalar_mul(out=sdiv256_col[:, cc:cc + 1],
                                    in0=s_col[:, cc:cc + 1],
                                    scalar1=1.0 / 256.0)
        # t = gn_bias - mean * s
        nb = small.tile([P, 1], F32, tag="gn_nb")
        nc.vector.tensor_scalar(
            out=nb, in0=e_sb[:, 0:1], scalar1=s_col[:, cc:cc + 1],
            scalar2=par["gn_bias"][:, cc:cc + 1],
            op0=ALU.mult, op1=ALU.subtract)
        t_f = small.tile([P, 1], F32, tag="gn_t")
        nc.vector.tensor_scalar_mul(out=t_f, in0=nb, scalar1=-1.0)
        nc.vector.tensor_copy(out=t_bf[:, cc:cc + 1], in_=t_f)

    gn_stat2 = [gn_chunk_stats(cc) for cc in range(CT)]

    # ---- PE program ------------------------------------------------------
    # 1) W3 = Wq^T Wk via fp8 DR (weights ship x16 -> psum = 256*W3).
    #    Evicted to bf16 right away (no stats dependency); the fp8 (x16,
    #    s_b row-scaled) copy for the R matmul follows once stats land.
    w38r = wpool.tile([P, CT, C], FP8, tag="w38r")
    w3bf = wpool.tile([P, CT, C], BF16, tag="w3bf")
    for bt in range(CT):
        ps = mm_ps.tile([P, C], F32, tag="mm", name=f"w3ps_{rep}_{bt}")
        for cop in (0, 2):
            nc.tensor.matmul(
                ps, lhsT=w_nat["wq"][:, cop:cop + 2, bt * P:(bt + 1) * P],
                rhs=w_nat["wk"][:, cop:cop + 2, :],
                start=(cop == 0), stop=(cop == 2), perf_mode=DR)
        nc.scalar.activation(out=w3bf[:, bt, :], in_=ps, func=AF.Copy,
                             scale=1.0 / 256.0)

    # 2) GroupNorm group-combine / expand matmuls (tiny).
    for cc in range(CT):
        g_ps = mm_ps.tile([GPC, 2], F32, tag="mm", name=f"gps_{rep}_{cc}")
        nc.tensor.matmul(g_ps, lhsT=gmat, rhs=gn_stat2[cc], start=True,
                         stop=True)
        grp = gn_chunk_finish(cc, g_ps)
        e_ps = mm_ps.tile([P, 2], F32, tag="mm", name=f"eps_{rep}_{cc}")
        nc.tensor.matmul(e_ps, lhsT=gexp, rhs=grp, start=True, stop=True)
        gn_chunk_expand(cc, e_ps)
    warm_e = small.tile([P, 1], F32, tag="warm", name=f"warm_e_{rep}")
    nc.scalar.activation(out=warm_e, in_=neg4, func=AF.Exp, bias=neg4)

    # fp8 W3 (x16, s_b row-scaled) once the stats are in.
    for bt in range(CT):
        nc.vector.tensor_scalar(out=w38r[:, bt, :], in0=w3bf[:, bt, :],
                                scalar1=s16_col[:, bt:bt + 1], scalar2=None,
                                op0=ALU.mult)

    # 3) swb[a] = s_a * (w3t[a] + w2[a]) = s_a * (W3^T t + Wk^T bq)[a]
    swb = consts.tile([P, CT], F32, tag="swb")
    for at in range(CT):
        ps = mm_ps.tile([P, 1], F32, tag="mm", name=f"swb_{rep}_{at}")
        for co in range(CT):
            nc.tensor.matmul(
                ps, lhsT=w_nat["wk"][:, co, at * P:(at + 1) * P],
                rhs=par["bq"][:, co:co + 1], start=(co == 0), stop=False)
        for bt in range(CT):
            nc.tensor.matmul(
                ps, lhsT=w3bf[:, bt, at * P:(at + 1) * P],
                rhs=t_bf[:, bt:bt + 1], start=False, stop=(bt == CT - 1))
        nc.vector.tensor_scalar(out=swb[:, at:at + 1], in0=ps,
                                scalar1=s_col[:, at:at + 1], scalar2=None,
                                op0=ALU.mult)

    # 4) R' for the first query chunk (the rest interleave into the loop).
    r8 = rpool.tile([P, CT, NQ], FP8, tag="r8")

    def emit_r(icq):
        for at in range(CT):
            ps = mm_ps.tile([P, NI], F32, tag="mm", name=f"r_{rep}_{icq}_{at}")
            for bcp in (0, 2):
                nc.tensor.matmul(
                    ps, lhsT=w38r[:, bcp:bcp + 2, at * P:(at + 1) * P],
                    rhs=x8[:, bcp:bcp + 2, icq * NI:(icq + 1) * NI],
                    start=(bcp == 0), stop=(bcp == 2), perf_mode=DR)
            nc.vector.tensor_scalar(
                out=r8[:, at, icq * NI:(icq + 1) * NI], in0=ps,
                scalar1=sdiv16_col[:, at:at + 1],
                scalar2=swb[:, at:at + 1], op0=ALU.mult, op1=ALU.add)

    emit_r(0)

    # 5) M2 = Wp Wv via fp8 DR (psum = 256*M2; rows = c_attn chunk, free =
    #    c_out). Evicted as fp8 with the s_a/256 scale -> m28 (proj lhsT)
    #    and bf16 true-scale -> m2bf. Then w4 = M2 t + Wp bv + bp.
    m28 = wpool.tile([P, CT, C], FP8, tag="m28")
    m2bf = wpool.tile([P, CT, C], BF16, tag="m2bf")
    for at in range(CT):
        ps = mm_ps.tile([P, C], F32, tag="mm", name=f"m2ps_{rep}_{at}")
        for ecp in (0, 2):
            nc.tensor.matmul(
                ps, lhsT=w_nat["wv"][:, ecp:ecp + 2, at * P:(at + 1) * P],
                rhs=w_nat["wp"][:, ecp:ecp + 2, :],
                start=(ecp == 0), stop=(ecp == 2), perf_mode=DR)
        nc.vector.tensor_scalar(out=m28[:, at, :], in0=ps,
                                scalar1=sdiv256_col[:, at:at + 1],
                                scalar2=None, op0=ALU.mult)
        nc.vector.tensor_scalar_mul(out=m2bf[:, at, :], in0=ps,
                                    scalar1=1.0 / 256.0)
    w4 = consts.tile([P, CT], F32, tag="w4")
    for dc in range(CT):
        ps = mm_ps.tile([P, 1], F32, tag="mm", name=f"w4_{rep}_{dc}")
        for ec in range(CT):
            nc.tensor.matmul(
                ps, lhsT=w_nat["wp"][:, ec, dc * P:(dc + 1) * P],
                rhs=par["bv"][:, ec:ec + 1], start=(ec == 0), stop=False)
        for at in range(CT):
            nc.tensor.matmul(
                ps, lhsT=m2bf[:, at, dc * P:(dc + 1) * P],
                rhs=t_bf[:, at:at + 1], start=False, stop=(at == CT - 1))
        nc.vector.tensor_add(out=w4[:, dc:dc + 1], in0=ps,
                             in1=par["bp"][:, dc:dc + 1])

    # ---- attention main loop ---------------------------------------------
    # The projection of chunk icq-1 is emitted INSIDE chunk icq's jp loop:
    # emitted at the boundary it sits between the loop and the next score
    # stream in PE program order and stalls ~6us on the DVE rec/eviction
    # chain while the ACT exp stream idles.
    def emit_proj(icq, att8):
        ob = outs.tile([P, CT, NI], FP16, tag="ob",
                       name=f"ob_{rep}_{icq}")
        for dc in range(CT):
            pp = mm_ps.tile([P, NI], F32, tag="mm",
                            name=f"pp_{rep}_{icq}_{dc}")
            for ctp in (0, 2):
                nc.tensor.matmul(
                    pp, lhsT=m28[:, ctp:ctp + 2, dc * P:(dc + 1) * P],
                    rhs=att8[:, ctp:ctp + 2, :],
                    start=(ctp == 0), stop=(ctp == 2), perf_mode=DR)
            nc.vector.scalar_tensor_tensor(
                out=ob[:, dc, :], in0=pp, scalar=w4[:, dc:dc + 1],
                in1=xbf[:, dc, icq * NI:(icq + 1) * NI],
                op0=ALU.add, op1=ALU.add)
        nc.sync.dma_start(
            out=out_d[:, icq * NI:(icq + 1) * NI].rearrange(
                "(t p) n -> p t n", p=P),
            in_=ob)

    pending = [None]
    for icq in range(IC):
        att_ps = [att_ps_pool.tile([P, NI], F32, tag="att",
                                   name=f"att_{rep}_{icq}_{ct}")
                  for ct in range(CT)]
        den_ps = den_ps_pool.tile([P, NI], F32, tag="den",
                                  name=f"den_{rep}_{icq}")
        for jp in range(JT // 2):
            e2 = epool.tile([P, 2, NI], FP8, tag="e",
                            name=f"e2_{rep}_{icq}_{jp}")
            for half in range(2):
                jc = jp * 2 + half
                s_ps = mm_ps.tile([P, NI], F32, tag="mm",
                                  name=f"s_{rep}_{icq}_{jc}")
                for acp in (0, 2):
                    nc.tensor.matmul(
                        s_ps, lhsT=x8[:, acp:acp + 2, jc * P:(jc + 1) * P],
                        rhs=r8[:, acp:acp + 2, icq * NI:(icq + 1) * NI],
                        start=(acp == 0), stop=(acp == 2), perf_mode=DR)
                nc.scalar.activation(out=e2[:, half, :], in_=s_ps,
                                     func=AF.Exp, scale=inv_sqrt_c,
                                     bias=neg4)
            for ct in range(CT):
                nc.tensor.matmul(
                    att_ps[ct],
                    lhsT=xt8[:, 2 * jp:2 * jp + 2, ct * P:(ct + 1) * P],
                    rhs=e2, start=(jp == 0), stop=(jp == JT // 2 - 1),
                    perf_mode=DR)
            nc.tensor.matmul(
                den_ps, lhsT=ones8, rhs=e2, start=(jp == 0),
                stop=(jp == JT // 2 - 1), perf_mode=DR)
            # previous chunk's projection: its att8 evictions finish on
            # DVE during these first jps, so proj runs without stalling PE
            if jp == 2 and pending[0] is not None:
                emit_proj(*pending[0])
                pending[0] = None
            # next query chunk's R lands mid-loop: PE and DVE both have
            # slack here, so the next score stream starts bubble-free
            if jp == 11 and icq + 1 < IC:
                emit_r(icq + 1)

        rec = outs.tile([P, NI], F32, tag="rec", bufs=2,
                        name=f"rec_{rep}_{icq}")
        nc.vector.reciprocal(out=rec, in_=den_ps)
        att8 = outs.tile([P, CT, NI], FP8, tag="attn", bufs=2,
                         name=f"att8_{rep}_{icq}")
        for ct in range(CT):
            nc.vector.tensor_mul(out=att8[:, ct, :], in0=att_ps[ct],
                                 in1=rec)
        pending[0] = (icq, att8)
    emit_proj(*pending[0])


def _build(repeat=1):
    nc = bacc.Bacc()
    x8_d = nc.declare_dram_parameter("x8", [C, N], FP8, isOutput=False)
    xt8_d = nc.declare_dram_parameter("xt8", [N, C], FP8, isOutput=False)
    xbf_d = nc.declare_dram_parameter("xbf", [C, NQ], FP16, isOutput=False)
    w_d = nc.declare_dram_parameter("w8all", [4 * C, C], FP8, isOutput=False)
    p_d = {p: nc.declare_dram_parameter(p, [C], F32, isOutput=False)
           for p in PARAM_NAMES}
    p_d.update({p: nc.declare_dram_parameter(p, [C], FP8, isOutput=False)
                for p in BIAS8_NAMES})
    out_d = nc.declare_dram_parameter("out", [C, NQ], FP16, isOutput=True)
    with tile.TileContext(nc) as tc, ExitStack() as ctx:
        _emit(ctx, nc, tc, x8_d, xt8_d, xbf_d, w_d, p_d, out_d,
              repeat=repeat)
    nc.finalize()
    return nc


def _get_nc():
    if "nc" not in _BUILD_CACHE:
        _BUILD_CACHE["nc"] = _build()
    return _BUILD_CACHE["nc"]


def _make_in_maps(x, gn_scale, gn_bias, wq, bq, wk, bk, wv, bv, wp, bp):
    xf = np.ascontiguousarray(np.asarray(x, dtype=np.float32).reshape(B, C, N))
    fp8 = ml_dtypes.float8_e4m3fn
    # weights ship fp8 scaled x16 (entries ~N(0, 1/C) would hit fp8's
    # subnormal range at scale 1) concatenated into one tensor (one DMA);
    # biases ship /16 so the F1 matmuls against x16 weights land at true
    # scale. wp ships pre-transposed: the kernel wants c_in on rows.
    w8all = np.concatenate([
        np.asarray(wq, np.float32) * 16.0,
        np.asarray(wk, np.float32) * 16.0,
        np.asarray(wv, np.float32) * 16.0,
        np.asarray(wp, np.float32).T * 16.0,
    ], axis=0).astype(fp8)
    shared = {
        "w8all": w8all,
        "bq": (np.asarray(bq, np.float32) / 16.0).astype(fp8),
        "bv": (np.asarray(bv, np.float32) / 16.0).astype(fp8),
        "bp": np.ascontiguousarray(np.asarray(bp, np.float32)),
        "gn_scale": np.ascontiguousarray(np.asarray(gn_scale, np.float32)),
        "gn_bias": np.ascontiguousarray(np.asarray(gn_bias, np.float32)),
    }
    in_maps = []
    for core in range(8):
        bi, qh = core // 2, core % 2
        xb = xf[bi]
        if qh == 0:
            xc = xb
        else:
            xc = np.ascontiguousarray(
                np.concatenate([xb[:, NQ:], xb[:, :NQ]], axis=1))
        x8 = xc.astype(fp8)
        xt8 = np.ascontiguousarray(xc.T).astype(fp8)
        xbf = np.ascontiguousarray(xc[:, :NQ]).astype(np.float16)
        in_maps.append({"x8": x8, "xt8": xt8, "xbf": xbf, **shared})
    return in_maps


def _gather(results):
    out = np.empty((B, C, N), np.float32)
    for core in range(8):
        bi, qh = core // 2, core % 2
        out[bi, :, qh * NQ:(qh + 1) * NQ] = \
            results[core]["out"].astype(np.float32)
    return out.reshape(B, C, HW, HW)


def kernel(x, gn_scale, gn_bias, wq, bq, wk, bk, wv, bv, wp, bp):
    nc = _get_nc()
    in_maps = _make_in_maps(x, gn_scale, gn_bias, wq, bq, wk, bk, wv, bv,
                            wp, bp)
    res = run_bass_kernel_spmd(nc, in_maps, core_ids=list(range(8)))
    return _gather(res.results)
